# revision 1
# baseline (speedup 1.0000x reference)
# AttnPool1dWindow Trainium2 kernel.
# B=8, C=512, L=4096, kernel_size=16, stride=8, P=511.
# Data-parallel: one batch per NeuronCore across 8 cores.
import numpy as np

B, C, L = 8, 512, 4096
WIN, ST = 16, 8
P = 1 + (L - WIN) // ST          # 511
P4 = 512                          # padded window count (last window dummy)
NEG = -1.0e9
NLC = 8                           # l-chunks of 512 tokens
NCT = 4                           # c tiles of 128
NDT = 4                           # d tiles of 128
NTT = 32                          # token tiles of 128
NPT = 4                           # p tiles of 128
LPAD = 8 * 528                    # srow length (= s8 rows * padded cols)

_CACHE = {}


def _build_host_constants():
    """Constant matrices shared by all cores (data independent)."""
    import ml_dtypes
    bf16 = ml_dtypes.bfloat16
    # E0[delta][r, q] = 1 iff q == 16*delta + r//8        (delta 0..7)
    # E1[delta][r, q] = 1 iff q == 16*delta + r//8 - 1    (delta 0..8)
    r = np.arange(128)
    q = np.arange(128)
    e0 = np.zeros((8, 128, 128), np.float32)
    for d in range(8):
        e0[d] = (q[None, :] == 16 * d + r[:, None] // 8).astype(np.float32)
    e1 = np.zeros((9, 128, 128), np.float32)
    for d in range(9):
        e1[d] = (q[None, :] == 16 * d + r[:, None] // 8 - 1).astype(np.float32)
    e0s = np.ascontiguousarray(e0.transpose(1, 0, 2).reshape(128, 8 * 128)).astype(bf16)
    e1s = np.ascontiguousarray(e1.transpose(1, 0, 2).reshape(128, 9 * 128)).astype(bf16)
    eye = np.eye(32, dtype=np.float32)
    return e0s, e1s, eye


def _custom_ap(ap, dims, extra_offset=0):
    """Copy of `ap` with explicit [step, count] dims (element units).

    NOTE: keep at most ONE non-mergeable free dim beyond the partition dim —
    the DMA lowering mis-steps middle dims of deeper APs.
    """
    import bass_rust
    c = ap.copy()
    c.ap = bass_rust.VecI64Pair(dims)
    if extra_offset:
        c.offset = c.offset + extra_offset
    return c


def _build_bass():
    import concourse.bacc as bacc
    import concourse.mybir as mybir
    import concourse.tile as tile
    dt = mybir.dt
    f32, bf16 = dt.float32, dt.bfloat16
    AF = mybir.ActivationFunctionType
    ALU = mybir.AluOpType

    nc = bacc.Bacc("TRN2", target_bir_lowering=False, debug=False, num_devices=8)

    # ---- DRAM I/O (per core) ----
    xbf_d = nc.declare_dram_parameter("xbf", [C, L], bf16, isOutput=False)
    xt_d = nc.declare_dram_parameter("xt", [L, C], bf16, isOutput=False)
    wt_d = nc.declare_dram_parameter("wtt", [C, C], bf16, isOutput=False)     # W^T [c,d]
    vemb_d = nc.declare_dram_parameter("vemb", [C, 64], bf16, isOutput=False)
    bcol_d = nc.declare_dram_parameter("bcol", [128, NDT], f32, isOutput=False)
    negm_d = nc.declare_dram_parameter("negm", [P4, WIN], f32, isOutput=False)
    notm_d = nc.declare_dram_parameter("notm", [P4, WIN], f32, isOutput=False)
    e0_d = nc.declare_dram_parameter("e0", [128, 8 * 128], bf16, isOutput=False)
    e1_d = nc.declare_dram_parameter("e1", [128, 9 * 128], bf16, isOutput=False)
    eye_d = nc.declare_dram_parameter("eye", [32, 32], f32, isOutput=False)
    out_d = nc.declare_dram_parameter("outt", [P4, C], f32, isOutput=True)    # out^T

    with tile.TileContext(nc) as tc:
        with (
            tc.tile_pool(name="big", bufs=1) as big,
            tc.tile_pool(name="hx", bufs=4) as hx,
            tc.tile_pool(name="smk", bufs=4) as smk,
            tc.tile_pool(name="outs", bufs=2) as outs,
            tc.tile_pool(name="rows", bufs=1, space="DRAM") as rows,
            tc.tile_pool(name="ypsum", bufs=2, space="PSUM") as ypsum,
            tc.tile_pool(name="spsum", bufs=1, space="PSUM") as spsum,
            tc.tile_pool(name="upsum", bufs=1, space="PSUM") as upsum,
            tc.tile_pool(name="opsum", bufs=2, space="PSUM") as opsum,
        ):
            # ---- resident SBUF tensors ----
            xbf = big.tile([128, NCT, L], bf16, tag="xbf")
            xt = big.tile([128, NTT, C], bf16, tag="xt")
            wtt = big.tile([128, NCT, C], bf16, tag="wtt")
            vemb = big.tile([128, NDT, 64], bf16, tag="vemb")
            bcol = big.tile([128, NDT], f32, tag="bcol")
            negm = big.tile([128, NPT, WIN], f32, tag="negm")
            notm = big.tile([128, NPT, WIN], f32, tag="notm")
            e0 = big.tile([128, 8 * 128], bf16, tag="e0")
            e1 = big.tile([128, 9 * 128], bf16, tag="e1")
            eye = big.tile([32, 32], f32, tag="eye")
            zt0 = big.tile([128, NTT, 512], bf16, tag="zt0")
            zt1 = big.tile([128, NTT, 512], bf16, tag="zt1")
            s8A = big.tile([5, 512], f32, tag="s8A")
            s8B = big.tile([3, 512], f32, tag="s8B")
            wtm = big.tile([128, NPT, WIN], f32, tag="wtm")
            u0chkA = big.tile([16, 128], f32, tag="u0chkA")
            u0chkB = big.tile([16, 128], f32, tag="u0chkB")
            u1chkA = big.tile([17, 128], f32, tag="u1chkA")
            u1chkB = big.tile([16, 128], f32, tag="u1chkB")
            u1chkbA = big.tile([17, 128], f32, tag="u1chkbA")
            u1chkbB = big.tile([16, 128], f32, tag="u1chkbB")
            ucol = big.tile([128, 64], f32, tag="ucol")
            sw_all = big.tile([128, NPT, WIN], f32, tag="sw_all")

            # DRAM scratch rows (token-ordered)
            srow = rows.tile([1, LPAD], f32, tag="srow")
            u0row = rows.tile([1, LPAD], f32, tag="u0row")
            u1row = rows.tile([1, LPAD], f32, tag="u1row")

            # ---- load inputs (2D APs only) ----
            for ci in range(NCT):
                nc.sync.dma_start(out=xbf[:, ci, :], in_=xbf_d[128 * ci:128 * ci + 128, :])
                nc.sync.dma_start(out=wtt[:, ci, :], in_=wt_d[128 * ci:128 * ci + 128, :])
            for di in range(NDT):
                nc.sync.dma_start(out=vemb[:, di, :], in_=vemb_d[128 * di:128 * di + 128, :])
            nc.sync.dma_start(out=bcol[:, :], in_=bcol_d[:, :])
            nc.sync.dma_start(
                out=negm[:, :, :],
                in_=_custom_ap(negm_d[:], [[WIN, 128], [128 * WIN, NPT], [1, WIN]]),
            )
            nc.sync.dma_start(
                out=notm[:, :, :],
                in_=_custom_ap(notm_d[:], [[WIN, 128], [128 * WIN, NPT], [1, WIN]]),
            )
            nc.sync.dma_start(out=eye[:, :], in_=eye_d[:, :])

            # zero u1row once: boundary chunk reads of not-yet-written tails
            # must see finite values (they hit zero-weight rows of E1).
            zrow = big.tile([1, LPAD], f32, tag="zrow")
            nc.vector.memset(zrow[0:1, :], 0.0)
            nc.sync.dma_start(out=u1row[0:1, :], in_=zrow[0:1, :])

            up_pool = upsum
            spA = spsum.tile([8, 512], f32, tag="SPA")
            spB = spsum.tile([8, 512], f32, tag="SPB")

            def phase1_chunk(li, sptile, first, last):
                for di in range(NDT):
                    yp = ypsum.tile([128, 512], f32, tag="Y")
                    for ci in range(NCT):
                        nc.tensor.matmul(
                            yp[:, :],
                            wtt[:, ci, 128 * di:128 * di + 128],
                            xbf[:, ci, 512 * li:512 * li + 512],
                            start=(ci == 0),
                            stop=(ci == NCT - 1),
                        )
                    h = hx.tile([128, 512], bf16, tag="H")
                    nc.scalar.activation(h[:, :], yp[:, :], AF.Tanh,
                                         bias=bcol[:, di:di + 1])
                    nc.tensor.matmul(
                        sptile[0:8, :],
                        vemb[:, di, 8 * li:8 * li + 8],
                        h[:, :],
                        start=(first and di == 0),
                        stop=(last and di == NDT - 1),
                    )

            def tail(half):
                # half 0: p-tiles 0..2 (tokens 0..2063); half 1: p-tiles 2..4
                ks = (0, 1) if half == 0 else (2, 3)
                if half == 0:
                    nc.scalar.copy(s8A[0:5, :], spA[0:5, :])
                    nc.sync.dma_start(out=srow[0:1, 0:2560], in_=s8A[0:5, :])
                else:
                    nc.scalar.copy(s8B[0:3, :], spB[0:3, :])
                    nc.sync.dma_start(out=srow[0:1, 2560:L], in_=s8B[0:3, :])
                    nc.sync.dma_start(out=srow[0:1, L:LPAD], in_=zrow[0:1, 0:128])
                for k in ks:
                    nc.sync.dma_start(
                        out=sw_all[:, k, :],
                        in_=_custom_ap(srow[:], [[LPAD, 1], [ST, 128], [1, WIN]],
                                       1024 * k),
                    )
                for k in ks:
                    swm = smk.tile([128, WIN], f32, tag="swm")
                    nc.vector.tensor_add(swm[:, :], sw_all[:, k, :], negm[:, k, :])
                    mx = smk.tile([128, 1], f32, tag="mx")
                    nc.vector.tensor_reduce(
                        mx[:, :], swm[:, :], axis=mybir.AxisListType.X, op=ALU.max,
                    )
                    mxn = smk.tile([128, 1], f32, tag="mxn")
                    nc.vector.tensor_scalar_mul(mxn[:, :], mx[:, :], -1.0)
                    ek = smk.tile([128, WIN], f32, tag="ek")
                    den = smk.tile([128, 1], f32, tag="den")
                    nc.scalar.activation(ek[:, :], swm[:, :], AF.Exp,
                                         bias=mxn[:, :], accum_out=den[:, :])
                    rden = smk.tile([128, 1], f32, tag="rden")
                    nc.vector.reciprocal(rden[:, :], den[:, :])
                    ewn = smk.tile([128, WIN], f32, tag="ewn")
                    nc.vector.tensor_mul(ewn[:, :], ek[:, :], notm[:, k, :])
                    nc.vector.tensor_scalar_mul(wtm[:, k, :], ewn[:, :], rden[:, :])
                    nc.sync.dma_start(
                        out=_custom_ap(u0row[:], [[LPAD, 1], [ST, 128], [1, 8]],
                                       1024 * k),
                        in_=wtm[:, k, 0:8],
                    )
                    nc.sync.dma_start(
                        out=_custom_ap(u1row[:], [[LPAD, 1], [ST, 128], [1, 8]],
                                       1024 * k + 8),
                        in_=wtm[:, k, 8:16],
                    )
                if half == 0:
                    nc.sync.dma_start(out=u0chkA[:, :], in_=u0row[0:1, 0:2048])
                    nc.sync.dma_start(out=u1chkA[:, :], in_=u1row[0:1, 0:2176])
                    nc.vector.tensor_copy(u1chkbA[:, :], u1chkA[:, :])
                    nc.vector.memset(u1chkbA[0:1, 0:8], 0.0)
                    upt = up_pool.tile([128, 33], f32, tag="UP")
                    nc.tensor.transpose(upt[:, 0:16], u0chkA[:, :], eye[0:16, 0:16])
                    nc.tensor.transpose(upt[:, 16:33], u1chkbA[:, :], eye[0:17, 0:17])
                    nc.vector.tensor_copy(ucol[:, 0:16], upt[:, 0:16])
                    nc.vector.tensor_copy(ucol[:, 32:49], upt[:, 16:33])
                    trng = range(0, 17)
                else:
                    nc.sync.dma_start(out=u0chkB[:, :], in_=u0row[0:1, 2048:L])
                    nc.sync.dma_start(out=u1chkB[:, :], in_=u1row[0:1, 2048:L])
                    nc.vector.tensor_copy(u1chkbB[:, :], u1chkB[:, :])
                    upt = up_pool.tile([128, 33], f32, tag="UP")
                    nc.tensor.transpose(upt[:, 0:16], u0chkB[:, :], eye[0:16, 0:16])
                    nc.tensor.transpose(upt[:, 16:32], u1chkbB[:, :], eye[0:16, 0:16])
                    nc.vector.tensor_copy(ucol[:, 16:32], upt[:, 0:16])
                    nc.vector.tensor_copy(ucol[:, 48:64], upt[:, 16:32])
                    trng = range(16, NTT)
                for t in trng:
                    if t < 32:
                        nc.vector.tensor_scalar_mul(zt0[:, t, :], xt[:, t, :],
                                                    ucol[:, t:t + 1])
                    nc.vector.tensor_scalar_mul(zt1[:, t, :], xt[:, t, :],
                                                ucol[:, 32 + t:32 + t + 1])
                for k in ks:
                    op = opsum.tile([128, 512], f32, tag="OP")
                    for d in range(8):
                        nc.tensor.matmul(
                            op[:, :], e0[:, 128 * d:128 * d + 128],
                            zt0[:, 8 * k + d, :],
                            start=(d == 0), stop=False,
                        )
                    for d in range(9):
                        t = 8 * k + d
                        if t >= NTT:
                            continue
                        nc.tensor.matmul(
                            op[:, :], e1[:, 128 * d:128 * d + 128], zt1[:, t, :],
                            start=False, stop=(d == 8 or t == NTT - 1),
                        )
                    ot = outs.tile([128, 512], f32, tag="OT")
                    nc.vector.tensor_copy(ot[:, :], op[:, :])
                    nc.sync.dma_start(out=out_d[128 * k:128 * k + 128, :],
                                        in_=ot[:, :])

            # ---- pipelined schedule: tail(0) overlaps phase-1 chunks 5..7 ----
            for li in range(5):
                phase1_chunk(li, spA, first=(li == 0), last=(li == 4))
            nc.sync.dma_start(out=e0[:, :], in_=e0_d[:, :])
            nc.sync.dma_start(out=e1[:, :], in_=e1_d[:, :])
            for tt in range(0, NTT, 8):
                nc.sync.dma_start(
                    out=xt[:, tt:tt + 8, :],
                    in_=_custom_ap(xt_d[:], [[C, 1], [C, 128], [128 * C, 8], [1, C]],
                                   128 * tt * C),
                )
            tail(0)
            for li in range(5, 8):
                phase1_chunk(li, spB, first=(li == 5), last=(li == 7))
            tail(1)
    nc.compile()
    return nc


def _prep_inputs(x, mask, W, b_, v):
    """Host-side shard prep: core i gets batch i."""
    import ml_dtypes
    bf16 = ml_dtypes.bfloat16
    e0s, e1s, eye = _build_host_constants()

    wtt = np.ascontiguousarray(W.T).astype(bf16)      # [c, d]
    vemb = np.zeros((C, 64), np.float32)
    for li in range(8):
        row = li if li < 5 else li - 5
        vemb[:, li * 8 + row] = v
    vemb = vemb.astype(bf16)
    bcol = np.ascontiguousarray(b_.reshape(NDT, 128).T).astype(np.float32)  # [128, 4]

    pidx = np.arange(P4)
    widx = np.arange(WIN)
    tok = pidx[:, None] * ST + widx[None, :]          # [P4, 16]
    valid = tok < L

    maps = []
    for bi in range(B):
        mw = np.ones((P4, WIN), bool)
        mw[valid] = mask[bi][tok[valid]]
        xb = x[bi]                                    # [C, L] fp32
        maps.append({
            "xbf": xb.astype(bf16),
            "xt": np.ascontiguousarray(xb.T).astype(bf16),
            "wtt": wtt,
            "vemb": vemb,
            "bcol": bcol,
            "negm": np.where(mw, np.float32(NEG), np.float32(0.0)),
            "notm": np.where(mw, np.float32(0.0), np.float32(1.0)),
            "e0": e0s,
            "e1": e1s,
            "eye": eye,
        })
    return maps


def kernel(x, mask, W, b, v):
    x = np.asarray(x, np.float32)
    mask = np.asarray(mask, bool)
    W = np.asarray(W, np.float32)
    b = np.asarray(b, np.float32)
    v = np.asarray(v, np.float32)

    from concourse.bass_utils import run_bass_kernel_spmd
    if "nc" not in _CACHE:
        _CACHE["nc"] = _build_bass()
    nc = _CACHE["nc"]

    in_maps = _prep_inputs(x, mask, W, b, v)
    res = run_bass_kernel_spmd(nc, in_maps, core_ids=list(range(8)))
    out = np.zeros((B, C, P), np.float32)
    for bi in range(B):
        outt = np.asarray(res.results[bi]["outt"], np.float32)   # [p, c]
        out[bi] = outt[:P].T
    return out


if __name__ == "__main__":
    import reference
    inputs = reference.setup_inputs()
    got = kernel(**{k: np.asarray(vv) for k, vv in inputs.items()})
    exp = np.asarray(reference.reference(**inputs))
    err = np.abs(got - exp).max() / np.abs(exp).max()
    print("scale-rel max err:", err)



# revision 9
# speedup vs baseline: 2.0112x; 2.0112x over previous
# AttnPool1dWindow Trainium2 kernel.
# B=8, C=512, L=4096, kernel_size=16, stride=8, P=511.
# Data-parallel: one batch per NeuronCore across 8 cores.
#
# Per-core pipeline (all on-chip; no DRAM round-trips in the softmax tail):
#   phase 1: y = (32*W) @ x in fp8 DoubleRow matmuls; h = tanh(y/32 + b) on
#            Act; s = v.h via fp16 matmul into per-chunk [1,512] psum rows.
#   tail   : PE-transpose score rows into a token-major [128,32] psum tile,
#            masked exp (token-major, free dim = col count), window sums /
#            reciprocal broadcasts via tiny selection matmuls, then the
#            pooling einsum as per-token-tile [128,16] matmuls accumulated
#            into one psum bank per output quarter.
import numpy as np

B, C, L = 8, 512, 4096
WIN, ST = 16, 8
P = 1 + (L - WIN) // ST          # 511
P4 = 512
NEG = -1.0e9
WSCALE = 32.0                    # W is scaled by this in fp8; tanh rescales

_CACHE = {}


def _custom_ap(ap, dims, extra_offset=0):
    """Copy of `ap` with explicit [step, count] dims (element units)."""
    import bass_rust
    c = ap.copy()
    c.ap = bass_rust.VecI64Pair(dims)
    if extra_offset:
        c.offset = c.offset + extra_offset
    return c


def _build_host_constants():
    import ml_dtypes
    f16 = np.float16
    r = np.arange(128)
    j = np.arange(16)
    i = np.arange(128)
    # blob16 [128, 68]: vemb(4) | M(16) | S0(16) | S1a(16) | S1b(16)
    M = (j[None, :] == r[:, None] // 8).astype(f16)          # [128,16]
    S0 = M
    S1a = (j[None, :] == r[:, None] // 8 - 1).astype(f16)    # r//8 == j+1
    S1b = ((j[None, :] == 15) & (r[:, None] // 8 == 0)).astype(f16)
    # blobR [16, 384]: R0 | R1a | R1b
    R0 = (i[None, :] // 8 == j[:, None]).astype(f16)         # [16,128]
    R1a = (i[None, :] // 8 == j[:, None] + 1).astype(f16)
    R1b = ((j[:, None] == 15) & (i[None, :] // 8 == 0)).astype(f16)
    blobR = np.concatenate([R0, R1a, R1b], axis=1)           # [16, 384]
    eye = np.eye(32, dtype=np.float32)
    return M, S0, S1a, S1b, blobR, eye


def _build_bass():
    import concourse.bacc as bacc
    import concourse.mybir as mybir
    import concourse.tile as tile
    dt = mybir.dt
    f32, f16, f8 = dt.float32, dt.float16, dt.float8e4
    AF = mybir.ActivationFunctionType
    PM = mybir.MatmulPerfMode

    nc = bacc.Bacc("TRN2", target_bir_lowering=False, debug=False, num_devices=8)

    # ---- DRAM I/O (per core) ----
    xbf8_d = nc.declare_dram_parameter("xbf8", [128, 4 * L], f8, isOutput=False)
    xt16_d = nc.declare_dram_parameter("xt16", [L, C], f16, isOutput=False)
    wtt8_d = nc.declare_dram_parameter("wtt8", [128, 4 * C], f8, isOutput=False)
    b16_d = nc.declare_dram_parameter("blob16", [128, 68], f16, isOutput=False)
    b32_d = nc.declare_dram_parameter("blob32", [128, 36], f32, isOutput=False)
    bR_d = nc.declare_dram_parameter("blobR", [16, 384], f32, isOutput=False)
    eye_d = nc.declare_dram_parameter("eyeT", [32, 32], f32, isOutput=False)
    out_d = nc.declare_dram_parameter("outd", [C, P4], f32, isOutput=True)

    with tile.TileContext(nc) as tc:
        with (
            tc.tile_pool(name="big", bufs=1) as big,
            tc.tile_pool(name="hx", bufs=2) as hx,
            tc.tile_pool(name="smp", bufs=2) as smp,
            tc.tile_pool(name="utp", bufs=8) as utp,
            tc.tile_pool(name="ucp", bufs=2) as ucp,
            tc.tile_pool(name="outs", bufs=2) as outsp,
            tc.tile_pool(name="yps", bufs=4, space="PSUM") as yps,
            tc.tile_pool(name="scps", bufs=1, space="PSUM") as scps,
            tc.tile_pool(name="dbps", bufs=1, space="PSUM") as dbps,
            tc.tile_pool(name="pops", bufs=2, space="PSUM") as pops,
        ):
            # ---- persistent SBUF ----
            wtt8 = big.tile([128, 4, C], f8, tag="wtt8")       # [c_lo, k, d]
            xbf8 = big.tile([128, 4, L], f8, tag="xbf8")       # [c_lo, k, t]
            xt16 = big.tile([128, 32, C], f16, tag="xt16")     # [t_lo, tau, c]
            blob16 = big.tile([128, 68], f16, tag="blob16")
            blob32 = big.tile([128, 36], f32, tag="blob32")
            blobR = big.tile([16, 384], f32, tag="blobR")
            eyeT = big.tile([32, 32], f32, tag="eyeT")
            e16 = big.tile([128, 33], f16, tag="e16")          # exp(s), col 32 = 0
            rden = big.tile([16, 33], f32, tag="rden")         # 1/den, col 0 = 0

            vemb = blob16[:, 0:4]          # [:, dt] = v (fp16)
            M_ = blob16[:, 4:20]
            S0 = blob16[:, 20:36]
            S1a = blob16[:, 36:52]
            S1b = blob16[:, 52:68]
            bcol = blob32[:, 0:4]          # bias per d-tile
            negm = blob32[:, 4:36]         # [128, 32] token-major -1e9 mask
            R0 = blobR[:, 0:128]
            R1a = blobR[:, 128:256]
            R1b = blobR[:, 256:384]

            # ---- psum (allocated once; written via single-start groups) ----
            scolP = scps.tile([128, 32], f32, tag="SCOL")      # token-major s
            dbt = dbps.tile([128, 2, 8], f32, tag="DB")        # den / rdenb

            # ---- const + input DMAs (issue order matters) ----
            nc.sync.dma_start(out=wtt8[:, :, :], in_=wtt8_d[:, :])

            def load_xbf(lq):
                for k in range(4):
                    nc.sync.dma_start(
                        out=xbf8[:, k, 1024 * lq:1024 * lq + 1024],
                        in_=_custom_ap(xbf8_d[:], [[4 * L, 128], [1, 1024]],
                                       L * k + 1024 * lq),
                    )

            def load_xt(tq):
                nc.sync.dma_start(
                    out=xt16[:, 8 * tq:8 * tq + 8, :],
                    in_=_custom_ap(xt16_d[:],
                                   [[C, 1], [C, 128], [128 * C, 8], [1, C]],
                                   1024 * tq * C),
                )

            load_xbf(0)
            nc.sync.dma_start(out=blob32[:, :], in_=b32_d[:, :])
            nc.sync.dma_start(out=blob16[:, :], in_=b16_d[:, :])
            load_xbf(1)
            nc.sync.dma_start(out=eyeT[:, :], in_=eye_d[:, :])
            nc.sync.dma_start(out=blobR[:, :], in_=bR_d[:, :])
            load_xt(0)
            load_xbf(2)
            load_xt(1)
            load_xbf(3)
            load_xt(2)
            load_xt(3)

            nc.vector.memset(e16[:, 32:33], 0.0)
            nc.vector.memset(rden[0:16, 0:1], 0.0)

            pop_tiles = {}

            def chunk(li):
                """phase-1 for tokens 512*li .. 512*li+511."""
                sp = spps.tile([1, 512], f32, tag="SP")
                h = hx.tile([128, 4, 512], f16, tag="H")
                for dtile in range(4):
                    y = yps.tile([128, 512], f32, tag="Y")
                    for cp in range(2):
                        nc.tensor.matmul(
                            y[:, :],
                            wtt8[:, 2 * cp:2 * cp + 2, 128 * dtile:128 * dtile + 128],
                            xbf8[:, 2 * cp:2 * cp + 2, 512 * li:512 * li + 512],
                            start=(cp == 0), stop=(cp == 1),
                            perf_mode=PM.DoubleRow,
                        )
                    nc.scalar.activation(h[:, dtile, :], y[:, :], AF.Tanh,
                                         bias=bcol[:, dtile:dtile + 1],
                                         scale=1.0 / WSCALE)
                    nc.tensor.matmul(sp[0:1, :], vemb[:, dtile:dtile + 1],
                                     h[:, dtile, :],
                                     start=(dtile == 0), stop=(dtile == 3))
                s8 = s8p.tile([1, 512], f32, tag="S8")
                nc.vector.tensor_copy(s8[0:1, :], sp[0:1, :])
                for c in range(4):
                    nc.tensor.matmul(
                        scolP[:, 4 * li + c:4 * li + c + 1],
                        s8[0:1, 128 * c:128 * c + 128],
                        eyeT[0:1, 0:1],
                        is_transpose=True,
                        start=(li == 0 and c == 0), stop=(li == 7 and c == 3),
                        skip_group_check=True,
                    )

            def flush_quarter(q):
                """Copy finished POP_q to SBUF and DMA to DRAM."""
                pop = pop_tiles.pop(q)
                ob = outsp.tile([128, 4, 128], f32, tag="OB")
                nc.vector.tensor_copy(ob[:, :, :], pop[:, :, :])
                nc.sync.dma_start(
                    out=_custom_ap(out_d[:], [[P4, 128], [128 * P4, 4], [1, 128]],
                                   128 * q),
                    in_=ob[:, :, :],
                )

            def tail(q):
                """softmax + pooling for output windows 128q .. 128q+127."""
                ncols = 9 if q < 3 else 8
                sm = smp.tile([128, 9], f32, tag="SM")
                nc.vector.tensor_add(sm[:, 0:ncols], scolP[:, 8 * q:8 * q + ncols],
                                     negm[:, 8 * q:8 * q + ncols])
                nc.scalar.activation(e16[:, 8 * q:8 * q + ncols], sm[:, 0:ncols],
                                     AF.Exp)
                # window sums -> dbt[0:16, 0, :]
                nc.tensor.matmul(dbt[0:16, 0, :], S0, e16[:, 8 * q:8 * q + 8],
                                 start=True, stop=False, skip_group_check=True)
                nc.tensor.matmul(dbt[0:16, 0, :], S1a, e16[:, 8 * q:8 * q + 8],
                                 start=False, stop=False, skip_group_check=True)
                nc.tensor.matmul(dbt[0:16, 0, :], S1b, e16[:, 8 * q + 1:8 * q + 9],
                                 start=False, stop=True, skip_group_check=True)
                dmx = ucp.tile([16, 8], f32, tag="DMX")
                nc.vector.tensor_scalar_max(dmx[0:16, :], dbt[0:16, 0, :], 1e-6)
                nc.vector.reciprocal(rden[0:16, 1 + 8 * q:9 + 8 * q], dmx[0:16, :])
                # broadcast reciprocals to token rows: dbt[:, 0, :] (u0), [:, 1, :] (u1)
                nc.tensor.matmul(dbt[:, 0, :], R0, rden[0:16, 1 + 8 * q:9 + 8 * q],
                                 start=True, stop=False, skip_group_check=True)
                nc.tensor.matmul(dbt[:, 1, :], R1a, rden[0:16, 1 + 8 * q:9 + 8 * q],
                                 start=False, stop=False, skip_group_check=True)
                nc.tensor.matmul(dbt[:, 1, :], R1b, rden[0:16, 8 * q:8 * q + 8],
                                 start=False, stop=True, skip_group_check=True)
                u0c = ucp.tile([128, 8], f32, tag="U0C")
                u1c = ucp.tile([128, 8], f32, tag="U1C")
                nc.vector.tensor_mul(u0c[:, :], e16[:, 8 * q:8 * q + 8], dbt[:, 0, :])
                nc.vector.tensor_mul(u1c[:, :], e16[:, 8 * q:8 * q + 8], dbt[:, 1, :])

                pop = pops.tile([128, 4, 128], f32, tag="POP")
                pop_tiles[q] = pop
                for k in range(8):
                    tau = 8 * q + k
                    ut = utp.tile([128, 2, 16], f16, tag="UT")
                    nc.vector.tensor_scalar_mul(ut[:, 0, :], M_, u0c[:, k:k + 1])
                    nc.vector.tensor_scalar_mul(ut[:, 1, :], M_, u1c[:, k:k + 1])
                    for ci in range(4):
                        xtt = xt16[:, tau, 128 * ci:128 * ci + 128]
                        nc.tensor.matmul(pop[:, ci, 16 * k:16 * k + 16],
                                         xtt, ut[:, 0, :],
                                         start=(k == 0 and ci == 0), stop=False,
                                         skip_group_check=True)
                        if k == 0:
                            if q > 0:
                                nc.tensor.matmul(
                                    pop_tiles[q - 1][:, ci, 127:128],
                                    xtt, ut[:, 1, 0:1],
                                    start=False, stop=False, skip_group_check=True,
                                )
                            nc.tensor.matmul(pop[:, ci, 0:15], xtt, ut[:, 1, 1:16],
                                             start=False, stop=False,
                                             skip_group_check=True)
                        else:
                            nc.tensor.matmul(
                                pop[:, ci, 16 * k - 1:16 * k + 15], xtt, ut[:, 1, :],
                                start=False,
                                stop=(k == 7 and ci == 3),
                                skip_group_check=True,
                            )
                if q > 0:
                    flush_quarter(q - 1)

            # ---- schedule: c0 c1 c2 T0 c3 c4 T1 c5 c6 T2 c7 T3 ----
            chunk(0)
            chunk(1)
            chunk(2)
            tail(0)
            chunk(3)
            chunk(4)
            tail(1)
            chunk(5)
            chunk(6)
            tail(2)
            chunk(7)
            tail(3)
            flush_quarter(3)
    nc.compile()
    return nc


def _prep_inputs(x, mask, W, b_, v):
    """Host-side shard prep: core i gets batch i."""
    import ml_dtypes
    f8 = ml_dtypes.float8_e4m3
    f16 = np.float16

    M, S0, S1a, S1b, blobR, eye = _build_host_constants()
    vemb = np.ascontiguousarray(v.reshape(4, 128).T)   # vemb[d_lo, dt]
    blob16 = np.concatenate([vemb, M, S0, S1a, S1b], axis=1).astype(f16)
    bcol = np.ascontiguousarray(b_.reshape(4, 128).T).astype(np.float32)

    # W^T scaled, fp8, DoubleRow layout [c_lo, k, d] flattened to [128, 4*C]
    wt = (W.T * WSCALE).astype(f8)                    # [c, d]
    wtt8 = np.ascontiguousarray(
        wt.reshape(4, 128, C).transpose(1, 0, 2)).reshape(128, 4 * C)

    maps = []
    for bi in range(B):
        xb = x[bi]                                    # [C, L] f32
        x8 = xb.astype(f8).reshape(4, 128, L).transpose(1, 0, 2).reshape(128, 4 * L)
        xt16 = np.ascontiguousarray(xb.T).astype(f16)  # [L, C]
        nm = np.where(mask[bi], np.float32(NEG), np.float32(0.0))  # [L]
        negm_b = np.ascontiguousarray(nm.reshape(32, 128).T)       # [128, 32]
        blob32 = np.concatenate([bcol, negm_b], axis=1).astype(np.float32)
        maps.append({
            "xbf8": x8,
            "xt16": xt16,
            "wtt8": wtt8,
            "blob16": blob16,
            "blob32": blob32,
            "blobR": blobR.astype(np.float32),
            "eyeT": eye,
        })
    return maps


def kernel(x, mask, W, b, v):
    x = np.asarray(x, np.float32)
    mask = np.asarray(mask, bool)
    W = np.asarray(W, np.float32)
    b = np.asarray(b, np.float32)
    v = np.asarray(v, np.float32)

    from concourse.bass_utils import run_bass_kernel_spmd
    if "nc" not in _CACHE:
        _CACHE["nc"] = _build_bass()
    nc = _CACHE["nc"]

    in_maps = _prep_inputs(x, mask, W, b, v)
    res = run_bass_kernel_spmd(nc, in_maps, core_ids=list(range(8)))
    out = np.zeros((B, C, P), np.float32)
    for bi in range(B):
        outd = np.asarray(res.results[bi]["outd"], np.float32)   # [C, P4]
        out[bi] = outd[:, :P]
    return out


if __name__ == "__main__":
    import reference
    inputs = reference.setup_inputs()
    got = kernel(**{k: np.asarray(vv) for k, vv in inputs.items()})
    exp = np.asarray(reference.reference(**inputs))
    err = np.abs(got - exp).max() / np.abs(exp).max()
    print("scale-rel max err:", err)
